# revision 11
# baseline (speedup 1.0000x reference)
"""Trainium2 Bass kernel for masked softmax attention returning (out, attn).

Problem: q,k,v [2,16,2048,64] f32, mask [2,2048] f32 (1 = masked key).
reference:
    scores = einsum(q,k)/8 + mask*-1e9 ; attn = softmax(scores, -1)
    out = attn @ v ; return (out, attn)

Sharding: B*H = 32 head-slabs, 4 per NeuronCore across 8 cores. No
cross-core communication; host splits inputs and concatenates outputs.

Per-slab device pipeline (all q/k tile indices are 128-row blocks):
  - PE-transpose Q,K into d-major [65, 2048] fp32r tiles; row 64 of K-tilde
    is mask*(-8e9) (so exp(0.125*(s-8e9)) == 0 on masked keys), row 64 of
    Q-tilde is 1.0.
  - per 512-query chunk: for each of 16 k-blocks: fp32r matmul gives
    S^T [128k, 512q] in PSUM; ACT exp(0.125*x) -> E^T fp32r in SBUF;
    fp32r matmul accumulates outT[65, 512] = V_aug^T @ E^T where V_aug has
    a ones column, so outT row 64 = softmax denominators.
  - per 128-query block: PE-transpose outT chunk -> [128, 65], DVE
    reciprocal of col 64, scale cols 0..63 -> out rows; PE-transpose E^T
    blocks back to q-major with the 1/rowsum normalize fused into the DVE
    PSUM->SBUF copy; DMA [128, 2048] attn row-blocks (8KB/partition).
"""

from collections import deque

import numpy as np

import concourse.bacc as bacc
import concourse.tile as tile
from concourse import mybir
from concourse.bass_utils import run_bass_kernel_spmd
from concourse.masks import make_identity

B, H, S, D = 2, 16, 2048, 64
N_CORES = 8
SLABS = (B * H) // N_CORES  # 4 head-slabs per core
NEGP = -8.0e9  # pre-scale mask bias; ACT exp applies 0.125 -> -1e9
F32 = mybir.dt.float32
F32R = mybir.dt.float32r
EXP = mybir.ActivationFunctionType.Exp

NT = S // 128  # 16 k-blocks / q-blocks
NQC = S // 512  # 4 query chunks

_nc = None
_last_results = None


def build_nc():
    nc = bacc.Bacc("TRN2")
    q_d = nc.declare_dram_parameter("q", [SLABS, S, D], F32, isOutput=False)
    k_d = nc.declare_dram_parameter("k", [SLABS, S, D], F32, isOutput=False)
    v_d = nc.declare_dram_parameter("v", [SLABS, S, D], F32, isOutput=False)
    m_d = nc.declare_dram_parameter("mneg", [SLABS, S], F32, isOutput=False)
    attn_d = nc.declare_dram_parameter("attn", [SLABS, S, S], F32, isOutput=True)
    out_d = nc.declare_dram_parameter("out", [SLABS, S, D], F32, isOutput=True)

    with tile.TileContext(nc) as tc:
        with (
            tc.tile_pool(name="consts", bufs=1) as consts,
            tc.tile_pool(name="loads", bufs=2) as loads,
            tc.tile_pool(name="til", bufs=2) as til,
            tc.tile_pool(name="etp", bufs=2 * NT) as etp,
            tc.tile_pool(name="stage", bufs=5) as stage,
            tc.tile_pool(name="ostage", bufs=2) as ostage,
            tc.tile_pool(name="otsb", bufs=2) as otsbp,
            tc.tile_pool(name="mrow", bufs=2) as mrowp,
            tc.tile_pool(name="small", bufs=6) as small,
            tc.tile_pool(name="psS", bufs=2, space="PSUM") as psS,
            tc.tile_pool(name="psO", bufs=2, space="PSUM") as psO,
            tc.tile_pool(name="psT", bufs=3, space="PSUM") as psT,
        ):
            ident = consts.tile([128, 128], F32)
            make_identity(nc, ident[:])
            ident_r = consts.tile([128, 128], F32R)
            nc.vector.tensor_copy(ident_r[:], ident[:])
            ones_vt = consts.tile([128, NT, 1], F32)
            nc.vector.memset(ones_vt[:], 1.0)
            ones_row = consts.tile([1, S], F32)
            nc.vector.memset(ones_row[:], 1.0)

            backlog = deque()

            def drain_backlog(n):
                for _ in range(min(n, len(backlog))):
                    backlog.popleft()()

            for s in range(SLABS):
                # ---- loads ----
                k_nat = loads.tile([128, NT, D], F32, tag="knat")
                nc.sync.dma_start(
                    k_nat[:], k_d[s].rearrange("(t p) d -> p t d", p=128)
                )
                q_nat = loads.tile([128, NT, D], F32, tag="qnat")
                nc.sync.dma_start(
                    q_nat[:], q_d[s].rearrange("(t p) d -> p t d", p=128)
                )
                v_nat = loads.tile([128, NT, D], F32, tag="vnat")
                nc.sync.dma_start(
                    v_nat[:], v_d[s].rearrange("(t p) d -> p t d", p=128)
                )
                mrow = mrowp.tile([1, S], F32, tag="mrow")
                nc.sync.dma_start(mrow[:], m_d[s : s + 1, :])

                # ---- V augmented with ones column, rounded to f32r ----
                v_aug = til.tile([128, NT, D + 1], F32R, tag="vaug")
                nc.vector.tensor_copy(v_aug[:, :, 0:D], v_nat[:])
                nc.vector.tensor_copy(v_aug[:, :, D : D + 1], ones_vt[:])

                # ---- K-tilde / Q-tilde: d-major [65, S] fp32r ----
                ktil = til.tile([D + 1, S], F32R, tag="ktil")
                qtil = til.tile([D + 1, S], F32R, tag="qtil")
                for nat, dst in ((k_nat, ktil), (q_nat, qtil)):
                    for g in range(NT // 4):
                        pt = psT.tile([D, 4, 128], F32, tag="pt")
                        for j in range(4):
                            nc.tensor.transpose(
                                pt[:, j, :], nat[:, g * 4 + j, :], ident[:]
                            )
                        nc.scalar.copy(
                            dst[0:D, g * 512 : (g + 1) * 512],
                            pt[:].rearrange("p a b -> p (a b)"),
                        )
                nc.vector.tensor_copy(ktil[D : D + 1, :], mrow[:])
                nc.vector.tensor_copy(qtil[D : D + 1, :], ones_row[:])

                out_sb = ostage.tile([128, NT, D], F32, tag="osb")

                for qc in range(NQC):
                    ets = [
                        etp.tile([128, 512], F32R, tag="et", name="et")
                        for _ in range(NT)
                    ]
                    otp = psO.tile([D + 1, 512], F32, tag="ot")
                    for kb in range(NT):
                        stp = psS.tile([128, 512], F32, tag="st")
                        nc.tensor.matmul(
                            stp[:],
                            ktil[:, kb * 128 : (kb + 1) * 128],
                            qtil[:, qc * 512 : (qc + 1) * 512],
                            start=True,
                            stop=True,
                        )
                        nc.scalar.activation(ets[kb][:], stp[:], EXP, scale=0.125)
                        if kb > 0:
                            nc.tensor.matmul(
                                otp[:],
                                v_aug[:, kb - 1, :],
                                ets[kb - 1][:],
                                start=(kb == 1),
                                stop=False,
                            )
                        # drain backlogged transpose/normalize work between
                        # matmuls so the PE's HAM clock gate never sees an
                        # idle (transpose-only) window and stays at 2.4 GHz.
                        drain_backlog(1 if kb % 4 else 2)
                    nc.tensor.matmul(
                        otp[:], v_aug[:, NT - 1, :], ets[NT - 1][:],
                        start=False, stop=True,
                    )
                    otsb = otsbp.tile([D + 1, 512], F32, tag="otsb")
                    nc.scalar.copy(otsb[:], otp[:])

                    def make_qb_head(qb, qc=qc, otsb=otsb, out_sb=out_sb):
                        rinv = small.tile([128, 1], F32, tag="rinv", name="rinv")

                        def emit():
                            qi = qc * 4 + qb
                            ofp = psT.tile([128, D + 1], F32, tag="pt", name="ofp")
                            nc.tensor.transpose(
                                ofp[:],
                                otsb[:, qb * 128 : (qb + 1) * 128],
                                ident[0 : D + 1, 0 : D + 1],
                            )
                            nc.vector.reciprocal(rinv[:], ofp[:, D : D + 1])
                            nc.vector.tensor_scalar_mul(
                                out_sb[:, qi, :], ofp[:, 0:D], rinv[:]
                            )

                        return emit, rinv

                    for qb in range(4):
                        head, rinv = make_qb_head(qb)
                        backlog.append(head)
                        astage = stage.tile([128, S], F32, tag="astage", name="astage")

                        def make_qb_group(
                            g, qb=qb, qc=qc, s=s, ets=ets, rinv=rinv, astage=astage
                        ):
                            def emit():
                                qi = qc * 4 + qb
                                etr = psT.tile(
                                    [128, 4, 128], F32R, tag="pt", name="etr"
                                )
                                for j in range(4):
                                    kb = g * 4 + j
                                    nc.tensor.transpose(
                                        etr[:, j, :],
                                        ets[kb][:, qb * 128 : (qb + 1) * 128],
                                        ident_r[:],
                                    )
                                nc.vector.tensor_scalar_mul(
                                    astage[:, g * 512 : (g + 1) * 512],
                                    etr[:]
                                    .rearrange("p a b -> p (a b)")
                                    .bitcast(F32),
                                    rinv[:],
                                )
                                if g == 3:
                                    nc.sync.dma_start(
                                        attn_d[s, qi * 128 : (qi + 1) * 128, :],
                                        astage[:],
                                    )

                            return emit

                        for g in range(4):
                            backlog.append(make_qb_group(g))

                def emit_out_dma(s=s, out_sb=out_sb):
                    def emit():
                        nc.sync.dma_start(
                            out_d[s].rearrange("(t p) d -> p t d", p=128), out_sb[:]
                        )

                    return emit

                backlog.append(emit_out_dma())
            drain_backlog(len(backlog))
    return nc


def _get_nc():
    global _nc
    if _nc is None:
        _nc = build_nc()
        _nc.compile()
    return _nc


def kernel(**inputs):
    q = np.ascontiguousarray(inputs["q"], dtype=np.float32).reshape(B * H, S, D)
    k = np.ascontiguousarray(inputs["k"], dtype=np.float32).reshape(B * H, S, D)
    v = np.ascontiguousarray(inputs["v"], dtype=np.float32).reshape(B * H, S, D)
    mask = np.asarray(inputs["mask"], dtype=np.float32)  # [B, S]
    mneg = mask * np.float32(NEGP)

    nc = _get_nc()
    in_maps = []
    for c in range(N_CORES):
        sl = slice(c * SLABS, (c + 1) * SLABS)
        bidx = np.arange(c * SLABS, (c + 1) * SLABS) // H
        in_maps.append(
            {
                "q": np.ascontiguousarray(q[sl]),
                "k": np.ascontiguousarray(k[sl]),
                "v": np.ascontiguousarray(v[sl]),
                "mneg": np.ascontiguousarray(mneg[bidx]),
            }
        )
    res = run_bass_kernel_spmd(nc, in_maps, core_ids=list(range(N_CORES)))
    global _last_results
    _last_results = res
    out = np.concatenate([r["out"] for r in res.results]).reshape(B, H, S, D)
    attn = np.concatenate([r["attn"] for r in res.results]).reshape(B, H, S, S)
    return out, attn


# revision 15
# speedup vs baseline: 1.0504x; 1.0504x over previous
"""Trainium2 Bass kernel for masked softmax attention returning (out, attn).

Problem: q,k,v [2,16,2048,64] f32, mask [2,2048] f32 (1 = masked key).
reference:
    scores = einsum(q,k)/8 + mask*-1e9 ; attn = softmax(scores, -1)
    out = attn @ v ; return (out, attn)

Sharding: B*H = 32 head-slabs, 4 per NeuronCore across 8 cores. No
cross-core communication; host splits inputs and concatenates outputs.

Per-slab device pipeline (all q/k tile indices are 128-row blocks):
  - PE-transpose Q,K into d-major [65, 2048] fp32r tiles; row 64 of K-tilde
    is mask*(-8e9) (so exp(0.125*(s-8e9)) == 0 on masked keys), row 64 of
    Q-tilde is 1.0.
  - per 512-query chunk: for each of 16 k-blocks: fp32r matmul gives
    S^T [128k, 512q] in PSUM; ACT exp(0.125*x) -> E^T fp32r in SBUF;
    fp32r matmul accumulates outT[65, 512] = V_aug^T @ E^T where V_aug has
    a ones column, so outT row 64 = softmax denominators.
  - per 128-query block: PE-transpose outT chunk -> [128, 65], DVE
    reciprocal of col 64, scale cols 0..63 -> out rows; PE-transpose E^T
    blocks back to q-major with the 1/rowsum normalize fused into the DVE
    PSUM->SBUF copy; DMA [128, 2048] attn row-blocks (8KB/partition).
"""

from collections import deque

import numpy as np

import concourse.bacc as bacc
import concourse.tile as tile
from concourse import mybir
from concourse.bass_utils import run_bass_kernel_spmd
from concourse.masks import make_identity

B, H, S, D = 2, 16, 2048, 64
N_CORES = 8
SLABS = (B * H) // N_CORES  # 4 head-slabs per core
NEGP = -8.0e9  # pre-scale mask bias; ACT exp applies 0.125 -> -1e9
F32 = mybir.dt.float32
F32R = mybir.dt.float32r
F16 = mybir.dt.float16
EXP = mybir.ActivationFunctionType.Exp

NT = S // 128  # 16 k-blocks / q-blocks
NQC = S // 512  # 4 query chunks

_nc = None
_last_results = None


def build_nc():
    nc = bacc.Bacc("TRN2")
    q_d = nc.declare_dram_parameter("q", [SLABS, S, D], F32, isOutput=False)
    k_d = nc.declare_dram_parameter("k", [SLABS, S, D], F32, isOutput=False)
    v_d = nc.declare_dram_parameter("v", [SLABS, S, D], F32, isOutput=False)
    m_d = nc.declare_dram_parameter("mneg", [SLABS, S], F32, isOutput=False)
    attn_d = nc.declare_dram_parameter("attn", [SLABS, S, S], F32, isOutput=True)
    out_d = nc.declare_dram_parameter("out", [SLABS, S, D], F32, isOutput=True)

    with tile.TileContext(nc) as tc:
        with (
            tc.tile_pool(name="consts", bufs=1) as consts,
            tc.tile_pool(name="loads", bufs=2) as loads,
            tc.tile_pool(name="til", bufs=2) as til,
            tc.tile_pool(name="etp", bufs=2 * NT) as etp,
            tc.tile_pool(name="stage", bufs=3) as stage,
            tc.tile_pool(name="ostage", bufs=2) as ostage,
            tc.tile_pool(name="otsb", bufs=2) as otsbp,
            tc.tile_pool(name="mrow", bufs=2) as mrowp,
            tc.tile_pool(name="small", bufs=6) as small,
            tc.tile_pool(name="psS", bufs=2, space="PSUM") as psS,
            tc.tile_pool(name="psO", bufs=2, space="PSUM") as psO,
            tc.tile_pool(name="psT", bufs=3, space="PSUM") as psT,
        ):
            ident = consts.tile([128, 128], F32)
            make_identity(nc, ident[:])
            ident_r = consts.tile([128, 128], F32R)
            nc.vector.tensor_copy(ident_r[:], ident[:])
            ident_h = consts.tile([128, 128], F16)
            nc.vector.tensor_copy(ident_h[:], ident[:])
            warm_f = consts.tile([128, 512], F32)
            nc.vector.memset(warm_f[:], 1.0)
            warm = consts.tile([128, 512], F32R)
            nc.vector.tensor_copy(warm[:], warm_f[:])
            wps = psS.tile([128, 512], F32, tag="st", name="warmps")
            for _ in range(18):
                nc.tensor.matmul(wps[:], ident_r[:], warm[:], start=True, stop=True)
            ones_vt = consts.tile([128, NT, 1], F32)
            nc.vector.memset(ones_vt[:], 1.0)
            ones_row = consts.tile([1, S], F32)
            nc.vector.memset(ones_row[:], 1.0)

            backlog = deque()

            def drain_backlog(n):
                for _ in range(min(n, len(backlog))):
                    backlog.popleft()()

            for s in range(SLABS):
                # ---- loads ----
                k_nat = loads.tile([128, NT, D], F32, tag="knat")
                nc.sync.dma_start(
                    k_nat[:], k_d[s].rearrange("(t p) d -> p t d", p=128)
                )
                q_nat = loads.tile([128, NT, D], F32, tag="qnat")
                nc.sync.dma_start(
                    q_nat[:], q_d[s].rearrange("(t p) d -> p t d", p=128)
                )
                v_nat = loads.tile([128, NT, D], F32, tag="vnat")
                nc.sync.dma_start(
                    v_nat[:], v_d[s].rearrange("(t p) d -> p t d", p=128)
                )
                mrow = mrowp.tile([1, S], F32, tag="mrow")
                nc.sync.dma_start(mrow[:], m_d[s : s + 1, :])

                # ---- V augmented with ones column, rounded to f32r ----
                v_aug = til.tile([128, NT, D + 1], F16, tag="vaug")
                nc.vector.tensor_copy(v_aug[:, :, 0:D], v_nat[:])
                nc.vector.tensor_copy(v_aug[:, :, D : D + 1], ones_vt[:])

                # ---- K-tilde / Q-tilde: d-major [65, S] fp32r ----
                ktil = til.tile([D + 1, S], F32R, tag="ktil")
                qtil = til.tile([D + 1, S], F32R, tag="qtil")
                for nat, dst in ((k_nat, ktil), (q_nat, qtil)):
                    for g in range(NT // 4):
                        pt = psT.tile([D, 4, 128], F32, tag="pt")
                        for j in range(4):
                            nc.tensor.transpose(
                                pt[:, j, :], nat[:, g * 4 + j, :], ident[:]
                            )
                        nc.scalar.copy(
                            dst[0:D, g * 512 : (g + 1) * 512],
                            pt[:].rearrange("p a b -> p (a b)"),
                        )
                nc.vector.tensor_copy(ktil[D : D + 1, :], mrow[:])
                nc.vector.tensor_copy(qtil[D : D + 1, :], ones_row[:])

                out_sb = ostage.tile([128, NT, D], F32, tag="osb")

                for qc in range(NQC):
                    ets = [
                        etp.tile([128, 512], F16, tag="et", name="et")
                        for _ in range(NT)
                    ]
                    otp = psO.tile([D + 1, 512], F32, tag="ot")
                    for kb in range(NT):
                        stp = psS.tile([128, 512], F32, tag="st")
                        nc.tensor.matmul(
                            stp[:],
                            ktil[:, kb * 128 : (kb + 1) * 128],
                            qtil[:, qc * 512 : (qc + 1) * 512],
                            start=True,
                            stop=True,
                        )
                        nc.scalar.activation(ets[kb][:], stp[:], EXP, scale=0.125)
                        if kb > 0:
                            nc.tensor.matmul(
                                otp[:],
                                v_aug[:, kb - 1, :],
                                ets[kb - 1][:],
                                start=(kb == 1),
                                stop=False,
                            )
                        # drain backlogged transpose/normalize work between
                        # matmuls so the PE's HAM clock gate never sees an
                        # idle (transpose-only) window and stays at 2.4 GHz.
                        drain_backlog(1 if kb % 4 else 2)
                    nc.tensor.matmul(
                        otp[:], v_aug[:, NT - 1, :], ets[NT - 1][:],
                        start=False, stop=True,
                    )
                    otsb = otsbp.tile([D + 1, 512], F32, tag="otsb")
                    nc.scalar.copy(otsb[:], otp[:])

                    def make_qb_head(qb, qc=qc, otsb=otsb, out_sb=out_sb):
                        rinv = small.tile([128, 1], F32, tag="rinv", name="rinv")

                        def emit():
                            qi = qc * 4 + qb
                            ofp = psT.tile([128, D + 1], F32, tag="pt", name="ofp")
                            nc.tensor.transpose(
                                ofp[:],
                                otsb[:, qb * 128 : (qb + 1) * 128],
                                ident[0 : D + 1, 0 : D + 1],
                            )
                            nc.vector.reciprocal(rinv[:], ofp[:, D : D + 1])
                            nc.vector.tensor_scalar_mul(
                                out_sb[:, qi, :], ofp[:, 0:D], rinv[:]
                            )

                        return emit, rinv

                    for qb in range(4):
                        head, rinv = make_qb_head(qb)
                        backlog.append(head)
                        if qb % 2 == 0:
                            astage2 = stage.tile(
                                [128, 2, S], F32, tag="astage", name="astage"
                            )
                        astage = astage2[:, qb % 2, :]

                        def make_qb_group(
                            g, qb=qb, qc=qc, s=s, ets=ets, rinv=rinv,
                            astage=astage, astage2=astage2,
                        ):
                            def emit():
                                qi = qc * 4 + qb
                                etr = psT.tile(
                                    [128, 4, 128], F16, tag="pt", name="etr"
                                )
                                for j in range(4):
                                    kb = g * 4 + j
                                    nc.tensor.transpose(
                                        etr[:, j, :],
                                        ets[kb][:, qb * 128 : (qb + 1) * 128],
                                        ident_h[:],
                                    )
                                nc.vector.tensor_scalar_mul(
                                    astage[:, g * 512 : (g + 1) * 512],
                                    etr[:].rearrange("p a b -> p (a b)"),
                                    rinv[:],
                                )
                                if g == 3 and qb % 2 == 1:
                                    q0 = (qi - 1) * 128
                                    dst = attn_d[s, q0 : q0 + 256, :].rearrange(
                                        "(a p) k -> p a k", p=128
                                    )
                                    nc.sync.dma_start(dst, astage2[:])

                            return emit

                        for g in range(4):
                            backlog.append(make_qb_group(g))

                def emit_out_dma(s=s, out_sb=out_sb):
                    def emit():
                        nc.sync.dma_start(
                            out_d[s].rearrange("(t p) d -> p t d", p=128), out_sb[:]
                        )

                    return emit

                backlog.append(emit_out_dma())
            drain_backlog(len(backlog))
    return nc


def _get_nc():
    global _nc
    if _nc is None:
        _nc = build_nc()
        _nc.compile()
    return _nc


def kernel(**inputs):
    q = np.ascontiguousarray(inputs["q"], dtype=np.float32).reshape(B * H, S, D)
    k = np.ascontiguousarray(inputs["k"], dtype=np.float32).reshape(B * H, S, D)
    v = np.ascontiguousarray(inputs["v"], dtype=np.float32).reshape(B * H, S, D)
    mask = np.asarray(inputs["mask"], dtype=np.float32)  # [B, S]
    mneg = mask * np.float32(NEGP)

    nc = _get_nc()
    in_maps = []
    for c in range(N_CORES):
        sl = slice(c * SLABS, (c + 1) * SLABS)
        bidx = np.arange(c * SLABS, (c + 1) * SLABS) // H
        in_maps.append(
            {
                "q": np.ascontiguousarray(q[sl]),
                "k": np.ascontiguousarray(k[sl]),
                "v": np.ascontiguousarray(v[sl]),
                "mneg": np.ascontiguousarray(mneg[bidx]),
            }
        )
    res = run_bass_kernel_spmd(nc, in_maps, core_ids=list(range(N_CORES)))
    global _last_results
    _last_results = res
    out = np.concatenate([r["out"] for r in res.results]).reshape(B, H, S, D)
    attn = np.concatenate([r["attn"] for r in res.results]).reshape(B, H, S, S)
    return out, attn
